# revision 17
# baseline (speedup 1.0000x reference)
"""Trainium2 Bass kernel for nn_MGCN: pairwise edge-MLP adjacency prediction +
dual-relation GCN stack, data-parallel over batch B=8 across 8 NeuronCores.

Per core (one sample):
  1. xaT/xbT = (x @ [wa|wb])^T via PE (x transposed on PE first, float32r).
  2. Edge pass: per group of 4 output rows, DVE (strips j=0..2) or ACT
     (strip j=3, issued ahead) computes t[(s,k), j] = relu(xb[j,k] +
     xa[4g+s,k] + eb1[k]) in bf16; PE contracts with a shifted
     block-diagonal w2 via accumulating matmuls at 32-partition col-strips.
  3. A-side work (A^T, degree d0, D0, L0^T) interleaves into the edge phase.
  4. Symmetrize e + e^T (PE transpose + DVE add), ACT exp -> Y = A_pred,
     then D1 / L1^T; GCN relation-0 matmuls start before L1 is ready.
  5. 3-layer GCN in transposed feature-major layout (hT [f, N], float32r),
     max-pool over nodes, final classifier.

All input-dependent compute runs on device; host only packs weights.
Inputs arrive via 5 merged DMAs (x, A, three const blobs).
"""
import numpy as np
import ml_dtypes

import concourse.bass as bass
import concourse.mybir as mybir
from concourse import tile
from concourse.bass_utils import run_bass_kernel_spmd

F32 = mybir.dt.float32
F32R = mybir.dt.float32r
BF16 = mybir.dt.bfloat16
AF = mybir.ActivationFunctionType
OP = mybir.AluOpType
AX = mybir.AxisListType

B, N, C, HID, F0, OUT = 8, 512, 256, 32, 64, 2
NT = N // 128  # 4 node tiles
EPS = 1e-5


def _legalize_waits(nc):
    """This toolchain's walrus accepts at most ONE sync wait per TPB
    instruction. Hoist every wait of a multi-wait instruction onto its own
    preceding same-engine InstNoOp."""
    n_fixed = 0
    for fn in nc.m.functions:
        for blk in fn.blocks:
            out = []
            for inst in blk.instructions:
                si = getattr(inst, "sync_info", None)
                if si is not None and len(si.on_wait) > 1:
                    for w in si.on_wait:
                        out.append(mybir.InstNoOp(
                            name=nc.get_next_instruction_name(),
                            engine=inst.engine, ins=[], outs=[],
                            sync_info=mybir.SyncInfo(on_wait=[w], on_update=[]),
                            bass_nofuse=True,
                        ))
                    inst.sync_info = mybir.SyncInfo(
                        on_wait=[], on_update=si.on_update
                    )
                    n_fixed += 1
                out.append(inst)
            blk.instructions = out
    return n_fixed


# const blob column layouts (host packing must match)
#   constf (f32):  ident 0:128 | ioffd 128:256 | actb 256:259 | eb2c 259:260
#                  | gb 260:263 | fcw 263:265 | fcb 265:266 | eb1 266:267
#                  | ones 267:268 (all-ones column; row 0 used as [1,128] via
#                  transpose-free trick below is NOT possible, so ones kept
#                  as its own [1,128] row: ones stored at rows 0, col 268)
CF = 397
#   constr (f32r): identr 0:128 | wab chunks 128:192, 192:256
#                  | gw0 chunks 256:320, 320:384, 384:448, 448:512
#                  | gw1 512:576 | gw2 576:640
CR = 640
#   constb (bf16): w2c[u] at 32u:32u+32 (u=0..7) | identb 256:384
CB = 384


def build_kernel():
    nc = bass.Bass(trn_type="TRN2")

    dx = nc.dram_tensor("x", [N, C], F32R, kind="ExternalInput")
    dA = nc.dram_tensor("A", [N, N], F32R, kind="ExternalInput")
    dcf = nc.dram_tensor("constf", [128, CF], F32, kind="ExternalInput")
    dcr = nc.dram_tensor("constr", [128, CR], F32R, kind="ExternalInput")
    dcb = nc.dram_tensor("constb", [128, CB], BF16, kind="ExternalInput")
    dout = nc.dram_tensor("out", [OUT, 1], F32, kind="ExternalOutput")

    with tile.TileContext(nc) as tc:
        with (
            tc.tile_pool(name="const", bufs=1) as cpool,
            tc.tile_pool(name="work", bufs=1) as wpool,
            tc.tile_pool(name="tt", bufs=16) as tpool,
            tc.tile_pool(name="ps", bufs=7, space="PSUM") as ppool,
            tc.tile_pool(name="ps2", bufs=1, space="PSUM") as ppool2,
        ):
            # ---------------- merged loads ----------------
            constr = cpool.tile([128, CR], F32R, name="constr", tag="constr")
            nc.sync.dma_start(constr[:], dcr[:])
            xbig = wpool.tile([128, NT, C], F32R, name="xbig", tag="xbig")
            xsrc = dx.rearrange("(t p) c -> p t c", p=128)
            nc.sync.dma_start(xbig[:, 0:2, :], xsrc[:, 0:2, :])
            nc.sync.dma_start(xbig[:, 2:4, :], xsrc[:, 2:4, :])
            constb = cpool.tile([128, CB], BF16, name="constb", tag="constb")
            nc.sync.dma_start(constb[:], dcb[:])
            constf = cpool.tile([128, CF], F32, name="constf", tag="constf")
            nc.sync.dma_start(constf[:], dcf[:])
            Abig = wpool.tile([128, NT, N], F32R, name="Abig", tag="Abig")
            nc.sync.dma_start(Abig[:], dA.rearrange("(t p) c -> p t c", p=128))

            ident = constf[:, 0:128]
            ioffd = constf[:, 128:256]
            actb = constf[:, 256:259]
            eb2c = constf[:, 259:260]
            gbt = constf[0:F0, 260:263]
            fcwt = constf[0:F0, 263:265]
            fcbt = constf[0:OUT, 265:266]
            eb1c = constf[0:HID, 266:267]
            onesrow = constf[0:1, 269:397]  # [1,128] of ones
            identr = constr[:, 0:128]
            wab = [constr[:, 128 + 64 * c:128 + 64 * (c + 1)] for c in range(2)]
            gw0t = [constr[:, 256 + 64 * q:256 + 64 * (q + 1)] for q in range(4)]
            gw1t = constr[:, 512:576]
            gw2t = constr[:, 576:640]
            w2cv = [constb[:, 32 * u:32 * (u + 1)] for u in range(8)]
            identb = constb[:, 256:384]
            xt = [xbig[:, t, :] for t in range(NT)]
            At = [Abig[:, t, :] for t in range(NT)]

            # ---------------- xaT / xbT ----------------
            xT = [wpool.tile([128, N], F32R, name=f"xT{c2}", tag=f"xT{c2}")
                  for c2 in range(2)]
            for c2 in range(2):
                pb = ppool.tile([128, N], F32R, name="bank", tag="bank")
                for t in range(NT):
                    nc.tensor.transpose(
                        pb[:, 128 * t:128 * (t + 1)],
                        xt[t][:, 128 * c2:128 * (c2 + 1)],
                        identr[:],
                    )
                nc.scalar.activation(xT[c2][:], pb[:], AF.Copy)
            abT = ppool.tile([2 * HID, N], F32, name="bank", tag="bank")
            for c2 in range(2):
                nc.tensor.matmul(
                    abT[:], wab[c2], xT[c2][:],
                    start=(c2 == 0), stop=(c2 == 1),
                )
            xaT = wpool.tile([HID, N], F32, name="xaT", tag="xaT")
            nc.vector.tensor_scalar(xaT[:], abT[0:HID, :], eb1c, None, OP.add)
            B_all = wpool.tile([128, 128], F32, name="B_all", tag="B_all")
            for s in range(4):
                nc.vector.tensor_copy(B_all[32 * s:32 * (s + 1), :], xaT[:, s::4])
            xbT_rep = wpool.tile([128, N], BF16, name="xbT_rep", tag="xbT_rep")
            nc.scalar.activation(xbT_rep[0:HID, :], abT[HID:2 * HID, :], AF.Copy)
            for s in range(1, 4):
                nc.scalar.activation(
                    xbT_rep[32 * s:32 * (s + 1), :], xbT_rep[0:HID, :], AF.Copy
                )

            # ---------------- edge pass (+ interleaved A-side) ----------------
            # strip j=3 relus on ACT (issued first per i-tile, so the in-order
            # PE queue never waits on a late ACT tile); j=0..2 on DVE.
            d_all = wpool.tile([128, 8], F32, name="d_all", tag="d_all")
            AhT = [wpool.tile([128, N], F32R, name=f"AhT{jt}", tag=f"AhT{jt}")
                   for jt in range(NT)]

            def a_side(jt):
                ap_ = ppool.tile([128, N], F32R, name="bank", tag="bank")
                for it2 in range(NT):
                    nc.tensor.transpose(
                        ap_[:, 128 * it2:128 * (it2 + 1)],
                        At[it2][:, 128 * jt:128 * (jt + 1)],
                        identr[:],
                    )
                nc.vector.tensor_scalar(
                    AhT[jt][:], ap_[:], 0.0, 0.0, OP.add, OP.add,
                    accum_out=d_all[:, jt:jt + 1],
                )
                nc.gpsimd.tensor_tensor(
                    AhT[jt][:, 128 * jt:128 * (jt + 1)],
                    AhT[jt][:, 128 * jt:128 * (jt + 1)],
                    ident, OP.add,
                )

            # A^T for two j-tiles while DVE builds the edge-bias tables
            # (fills the PE bubble before the first edge matmul)
            a_side(0)
            a_side(1)

            lnd = wpool.tile([128, 8], F32, name="lnd", tag="lnd")
            D_all = wpool.tile([128, 8], F32, name="D_all", tag="D_all")
            Dt = wpool.tile([1, 8, 128], F32, name="Dt", tag="Dt")
            Dbc = [wpool.tile([128, N], F32, name=f"Dbc{r}", tag=f"Dbc{r}")
                   for r in range(2)]

            def d_path(r):
                lo = 4 * r
                nc.scalar.activation(lnd[:, lo:lo + 4], d_all[:, lo:lo + 4],
                                     AF.Ln, bias=actb[:, r:r + 1])
                nc.scalar.activation(D_all[:, lo:lo + 4], lnd[:, lo:lo + 4],
                                     AF.Exp, bias=actb[:, 2:3], scale=-0.5)
                dtp = ppool2.tile([1, 4, 128], F32, name="small", tag="small")
                for c4 in range(4):
                    nc.tensor.transpose(
                        dtp[0:1, c4, :], D_all[:, lo + c4:lo + c4 + 1], ident
                    )
                nc.vector.tensor_copy(Dt[0:1, lo:lo + 4, :], dtp[:])
                bp = ppool.tile([128, N], F32, name="bank", tag="bank")
                for jt in range(NT):
                    nc.tensor.matmul(
                        bp[:, 128 * jt:128 * (jt + 1)],
                        onesrow, Dt[0:1, lo + jt, :],
                        start=True, stop=True,
                    )
                nc.scalar.activation(Dbc[r][:], bp[:], AF.Copy)

            def edge_relus_j3(it):
                tiles = []
                for u in range(8):
                    g = 32 * it + 8 * 3 + u
                    t_a = tpool.tile([128, N], BF16, name="ta", tag="ta")
                    nc.scalar.activation(t_a[:], xbT_rep[:], AF.Relu,
                                         bias=B_all[:, g:g + 1])
                    tiles.append(t_a)
                return tiles

            L0T = [wpool.tile([128, N], F32R, name=f"L0T{jt}", tag=f"L0T{jt}")
                   for jt in range(NT)]
            L1T = [wpool.tile([128, N], F32R, name=f"L1T{jt}", tag=f"L1T{jt}")
                   for jt in range(NT)]
            e_sb = [wpool.tile([128, N], BF16, name=f"e{it}", tag=f"e{it}")
                    for it in range(NT)]
            s_sb = [wpool.tile([128, N], BF16, name=f"s{it}", tag=f"s{it}")
                    for it in range(NT)]
            etp = [None] * NT

            def sym_chunk(k):
                # e_sb[k] just landed: transpose it into column k of every
                # etp row, and do every e+e^T chunk add that is now ready
                for it2 in range(NT):
                    if etp[it2] is None:
                        etp[it2] = ppool.tile([128, N], BF16, name="etp",
                                              tag="bank")
                    nc.tensor.transpose(
                        etp[it2][:, 128 * k:128 * (k + 1)],
                        e_sb[k][:, 128 * it2:128 * (it2 + 1)],
                        identb,
                    )
                for it2 in range(k + 1):
                    nc.vector.tensor_tensor(
                        s_sb[it2][:, 128 * k:128 * (k + 1)],
                        e_sb[it2][:, 128 * k:128 * (k + 1)],
                        etp[it2][:, 128 * k:128 * (k + 1)], OP.add,
                    )
                for jt2 in range(k):
                    nc.vector.tensor_tensor(
                        s_sb[k][:, 128 * jt2:128 * (jt2 + 1)],
                        e_sb[k][:, 128 * jt2:128 * (jt2 + 1)],
                        etp[k][:, 128 * jt2:128 * (jt2 + 1)], OP.add,
                    )

            eps = [None] * NT
            j3 = edge_relus_j3(0)
            for it in range(NT):
                ep = ppool.tile([128, N], F32, name="bank", tag="bank")
                eps[it] = ep
                for u in range(8):
                    for j in range(4):
                        g = 32 * it + 8 * j + u
                        if j == 3:
                            t_t = j3[u]
                        else:
                            t_t = tpool.tile([128, N], BF16, name="t", tag="t")
                            nc.vector.tensor_scalar(
                                t_t[:], xbT_rep[:], B_all[:, g:g + 1], 0.0,
                                OP.add, OP.max,
                            )
                        nc.tensor.matmul(
                            ep[32 * j:32 * (j + 1), :], w2cv[u], t_t[:],
                            start=(u == 0), stop=(u == 7),
                            tile_position=(0, 32 * j),
                        )
                # next i-tile's ACT relus BEFORE this tile's e_sb copy, so
                # the in-order ACT queue never stalls the next PE strip
                if it + 1 < NT:
                    j3 = edge_relus_j3(it + 1)
                nc.scalar.activation(e_sb[it][:], eps[it][:], AF.Copy)
                sym_chunk(it)
                if it < 2:
                    a_side(2 + it)
                if it == 2:
                    # D0 chain mid-edge: its ACT ln/exp slots into the edge
                    # ACT stream here instead of queueing behind all of it;
                    # L0T built on the idle GPSIMD engine
                    d_path(0)
                    for jt in range(NT):
                        nc.vector.scalar_tensor_tensor(
                            L0T[jt][:], AhT[jt][:], D_all[:, jt:jt + 1],
                            Dbc[0][:], OP.mult, OP.mult,
                        )

            # ---------------- exp -> Y = A_pred, d1 ----------------
            Yt = [wpool.tile([128, N], F32, name=f"Y{it}", tag=f"Y{it}")
                  for it in range(NT)]
            d1raw = wpool.tile([128, 4], F32, name="d1raw", tag="d1raw")
            diagv = wpool.tile([128, 4], F32, name="diagv", tag="diagv")
            for it in range(NT):
                nc.scalar.activation(Yt[it][:], s_sb[it][:], AF.Exp, bias=eb2c,
                                     scale=0.5, accum_out=d1raw[:, it:it + 1])
                # extract the (wrong) diagonal, fix the diag block to
                # A_hat1 = Y_offdiag + I, and correct d1 = d1raw - diag
                # (the +1 from I is folded into the Ln bias)
                dblk = Yt[it][:, 128 * it:128 * (it + 1)]
                scr = tpool.tile([128, 128], F32, name="scr", tag="scr", bufs=2)
                nc.vector.scalar_tensor_tensor(
                    scr[:], dblk, 1.0, ident, OP.mult, OP.mult,
                    accum_out=diagv[:, it:it + 1],
                )
                nc.vector.tensor_tensor(dblk, dblk, scr[:], OP.subtract)
                nc.vector.tensor_tensor(dblk, dblk, ident, OP.add)
                nc.vector.tensor_tensor(
                    d_all[:, 4 + it:5 + it], d1raw[:, it:it + 1],
                    diagv[:, it:it + 1], OP.subtract,
                )

            # ---------------- GCN layer 0, relation 0 (overlaps L1T build) --
            xh0 = [wpool.tile([128, N], F32R, name=f"xh0_{q}", tag=f"xh0_{q}")
                   for q in range(4)]
            up0 = []
            for c2 in range(2):
                up = ppool.tile([128, N], F32, name="bank", tag="bank")
                for jt in range(NT):
                    nc.tensor.matmul(
                        up[:], xt[jt][:, 128 * c2:128 * (c2 + 1)], L0T[jt][:],
                        start=(jt == 0), stop=(jt == NT - 1),
                    )
                nc.scalar.activation(xh0[c2][:], up[:], AF.Copy)

            d_path(1)
            for jt in range(NT):
                nc.vector.scalar_tensor_tensor(
                    L1T[jt][:], Yt[jt][:], D_all[:, 4 + jt:5 + jt], Dbc[1][:],
                    OP.mult, OP.mult,
                )

            # ---------------- GCN layer 0, relation 1 + dense ----------------
            for c2 in range(2):
                up = ppool.tile([128, N], F32, name="bank", tag="bank")
                for jt in range(NT):
                    nc.tensor.matmul(
                        up[:], xt[jt][:, 128 * c2:128 * (c2 + 1)], L1T[jt][:],
                        start=(jt == 0), stop=(jt == NT - 1),
                    )
                nc.scalar.activation(xh0[2 + c2][:], up[:], AF.Copy)
            zp = ppool.tile([F0, N], F32, name="bank", tag="bank")
            for q in range(4):
                nc.tensor.matmul(
                    zp[:], gw0t[q], xh0[q][:],
                    start=(q == 0), stop=(q == 3),
                )
            h1T = wpool.tile([F0, N], F32R, name="h1T", tag="h1T")
            nc.scalar.activation(h1T[:], zp[:], AF.Relu, bias=gbt[:, 0:1])

            # ---------------- GCN layers 1, 2 ----------------
            hT = h1T
            for li in (1, 2):
                h_sb = [wpool.tile([128, F0], F32R, name=f"h{li}_{jt}",
                                   tag=f"h{li}_{jt}") for jt in range(NT)]
                for jt in range(NT):
                    hp = ppool2.tile([128, F0], F32R, name="small", tag="small")
                    nc.tensor.transpose(
                        hp[:], hT[:, 128 * jt:128 * (jt + 1)],
                        identr[0:F0, 0:F0],
                    )
                    nc.vector.tensor_copy(h_sb[jt][:], hp[:])
                xh1 = wpool.tile([128, N], F32R, name=f"xh{li}", tag=f"xh{li}")
                for r in range(2):
                    up = ppool.tile([F0, N], F32, name="bank", tag="bank")
                    LrT = L0T if r == 0 else L1T
                    for jt in range(NT):
                        nc.tensor.matmul(
                            up[:], h_sb[jt][:], LrT[jt][:],
                            start=(jt == 0), stop=(jt == NT - 1),
                        )
                    nc.scalar.activation(xh1[F0 * r:F0 * (r + 1), :], up[:],
                                         AF.Copy)
                zp2 = ppool.tile([F0, N], F32, name="bank", tag="bank")
                gwt = gw1t if li == 1 else gw2t
                nc.tensor.matmul(zp2[:], gwt, xh1[:], start=True, stop=True)
                houtT = wpool.tile([F0, N], F32R, name=f"h{li}T", tag=f"h{li}T")
                nc.scalar.activation(houtT[:], zp2[:], AF.Relu,
                                     bias=gbt[:, li:li + 1])
                hT = houtT

            # ---------------- pool + classifier ----------------
            gmax = wpool.tile([F0, 1], F32, name="gmax", tag="gmax")
            nc.vector.tensor_reduce(gmax[:], hT[:], axis=AX.X, op=OP.max)
            op_ = ppool2.tile([OUT, 1], F32, name="small", tag="small")
            nc.tensor.matmul(op_[:], fcwt, gmax[:], start=True, stop=True)
            obuf = wpool.tile([OUT, 1], F32, name="obuf", tag="obuf")
            nc.vector.tensor_scalar(obuf[:], op_[:], fcbt, None, OP.add)
            nc.sync.dma_start(dout[:], obuf[:])

    _legalize_waits(nc)
    nc.finalize()
    return nc


_CACHE: dict = {}


def _get_compiled():
    if "nc" not in _CACHE:
        _CACHE["nc"] = build_kernel()
    return _CACHE["nc"]


def _pack_consts(ew1, eb1, ew2, eb2, gw0, gb0, gw1, gb1, gw2, gb2, fcw, fcb):
    bf = ml_dtypes.bfloat16
    f32 = np.float32
    ident = np.eye(128, dtype=f32)

    constf = np.zeros((128, CF), f32)
    constf[:, 0:128] = ident
    constf[:, 128:256] = 1.0 - ident
    constf[:, 256] = 1.0 + EPS
    constf[:, 257] = 1.0 + EPS
    constf[:, 258] = 0.0
    constf[:, 259] = float(np.asarray(eb2).reshape(-1)[0])
    constf[0:F0, 260] = np.asarray(gb0, f32)
    constf[0:F0, 261] = np.asarray(gb1, f32)
    constf[0:F0, 262] = np.asarray(gb2, f32)
    constf[0:F0, 263:265] = np.asarray(fcw, f32)
    constf[0:OUT, 265] = np.asarray(fcb, f32).reshape(-1)
    constf[0:HID, 266] = np.asarray(eb1, f32)
    constf[:, 267] = 1.0
    constf[:, 268] = 1.0
    constf[0, 269:397] = 1.0

    constr = np.zeros((128, CR), f32)
    constr[:, 0:128] = ident
    wabf = np.concatenate([ew1[:C], ew1[C:]], axis=1).astype(f32)  # [256, 64]
    constr[:, 128:192] = wabf[0:128]
    constr[:, 192:256] = wabf[128:256]
    g0 = np.asarray(gw0, f32)
    for q in range(4):
        constr[:, 256 + 64 * q:256 + 64 * (q + 1)] = g0[128 * q:128 * (q + 1)]
    constr[:, 512:576] = np.asarray(gw1, f32)
    constr[:, 576:640] = np.asarray(gw2, f32)

    constb = np.zeros((128, CB), f32)
    w2 = np.asarray(ew2[:, 0], f32)
    for s in range(4):
        for u in range(8):
            constb[32 * s:32 * (s + 1), 32 * u + 4 * u + s] = w2
    constb[:, 256:384] = ident
    return dict(
        constf=constf,
        constr=constr,
        constb=constb.astype(bf),
    )


def kernel(x, A, mask, ew1, eb1, ew2, eb2, gw0, gb0, gw1, gb1, gw2, gb2,
           fcw, fcb, _trace=False, **_ignored):
    nc = _get_compiled()
    consts = _pack_consts(ew1, eb1, ew2, eb2, gw0, gb0, gw1, gb1, gw2, gb2,
                          fcw, fcb)
    in_maps = [
        dict(
            consts,
            x=np.ascontiguousarray(np.asarray(x)[b], np.float32),
            A=np.ascontiguousarray(np.asarray(A)[b], np.float32),
        )
        for b in range(B)
    ]
    res = run_bass_kernel_spmd(
        nc, in_maps, core_ids=list(range(B)), trace=_trace
    )
    out = np.stack(
        [res.results[b]["out"].reshape(OUT) for b in range(B)]
    ).astype(np.float32)
    if _trace:
        kernel.last_results = res
    return out


# revision 18
# speedup vs baseline: 1.1434x; 1.1434x over previous
"""Trainium2 Bass kernel for nn_MGCN: pairwise edge-MLP adjacency prediction +
dual-relation GCN stack, data-parallel over batch B=8 across 8 NeuronCores.

Per core (one sample):
  1. xaT/xbT = (x @ [wa|wb])^T via PE (x transposed on PE first, float32r).
  2. Edge pass: per group of 4 output rows, DVE (strips j=0..2) or ACT
     (strip j=3, issued ahead) computes t[(s,k), j] = relu(xb[j,k] +
     xa[4g+s,k] + eb1[k]) in bf16; PE contracts with a shifted
     block-diagonal w2 via accumulating matmuls at 32-partition col-strips.
  3. A-side work (A^T, degree d0, D0, L0^T) interleaves into the edge phase.
  4. Symmetrize e + e^T (PE transpose + DVE add), ACT exp -> Y = A_pred,
     then D1 / L1^T; GCN relation-0 matmuls start before L1 is ready.
  5. 3-layer GCN in transposed feature-major layout (hT [f, N], float32r),
     max-pool over nodes, final classifier.

All input-dependent compute runs on device; host only packs weights.
Inputs arrive via 5 merged DMAs (x, A, three const blobs).
"""
import numpy as np
import ml_dtypes

import concourse.bass as bass
import concourse.mybir as mybir
from concourse import tile
from concourse.bass_utils import run_bass_kernel_spmd

F32 = mybir.dt.float32
F32R = mybir.dt.float32r
BF16 = mybir.dt.bfloat16
AF = mybir.ActivationFunctionType
OP = mybir.AluOpType
AX = mybir.AxisListType

B, N, C, HID, F0, OUT = 8, 512, 256, 32, 64, 2
NT = N // 128  # 4 node tiles
EPS = 1e-5


def _legalize_waits(nc):
    """This toolchain's walrus accepts at most ONE sync wait per TPB
    instruction. Hoist every wait of a multi-wait instruction onto its own
    preceding same-engine InstNoOp."""
    n_fixed = 0
    for fn in nc.m.functions:
        for blk in fn.blocks:
            out = []
            for inst in blk.instructions:
                si = getattr(inst, "sync_info", None)
                if si is not None and len(si.on_wait) > 1:
                    for w in si.on_wait:
                        out.append(mybir.InstNoOp(
                            name=nc.get_next_instruction_name(),
                            engine=inst.engine, ins=[], outs=[],
                            sync_info=mybir.SyncInfo(on_wait=[w], on_update=[]),
                            bass_nofuse=True,
                        ))
                    inst.sync_info = mybir.SyncInfo(
                        on_wait=[], on_update=si.on_update
                    )
                    n_fixed += 1
                out.append(inst)
            blk.instructions = out
    return n_fixed


# const blob column layouts (host packing must match)
#   constf (f32):  ident 0:128 | ioffd 128:256 | actb 256:259 | eb2c 259:260
#                  | gb 260:263 | fcw 263:265 | fcb 265:266 | eb1 266:267
#                  | ones 267:268 (all-ones column; row 0 used as [1,128] via
#                  transpose-free trick below is NOT possible, so ones kept
#                  as its own [1,128] row: ones stored at rows 0, col 268)
CF = 397
#   constr (f32r): identr 0:128 | wab chunks 128:192, 192:256
#                  | gw0 chunks 256:320, 320:384, 384:448, 448:512
#                  | gw1 512:576 | gw2 576:640
CR = 640
#   constb (bf16): w2c[u] at 32u:32u+32 (u=0..7) | identb 256:384
CB = 384


def build_kernel():
    nc = bass.Bass(trn_type="TRN2")

    dx = nc.dram_tensor("x", [N, C], F32R, kind="ExternalInput")
    dA = nc.dram_tensor("A", [N, N], F32R, kind="ExternalInput")
    dcf = nc.dram_tensor("constf", [128, CF], F32, kind="ExternalInput")
    dcr = nc.dram_tensor("constr", [128, CR], F32R, kind="ExternalInput")
    dcb = nc.dram_tensor("constb", [128, CB], BF16, kind="ExternalInput")
    dout = nc.dram_tensor("out", [OUT, 1], F32, kind="ExternalOutput")

    with tile.TileContext(nc) as tc:
        with (
            tc.tile_pool(name="const", bufs=1) as cpool,
            tc.tile_pool(name="work", bufs=1) as wpool,
            tc.tile_pool(name="tt", bufs=16) as tpool,
            tc.tile_pool(name="ps", bufs=6, space="PSUM") as ppool,
            tc.tile_pool(name="ps2", bufs=2, space="PSUM") as ppool2,
        ):
            # ---------------- merged loads ----------------
            constr = cpool.tile([128, CR], F32R, name="constr", tag="constr")
            nc.sync.dma_start(constr[:], dcr[:])
            xbig = wpool.tile([128, NT, C], F32R, name="xbig", tag="xbig")
            xsrc = dx.rearrange("(t p) c -> p t c", p=128)
            nc.sync.dma_start(xbig[:, 0:2, :], xsrc[:, 0:2, :])
            nc.sync.dma_start(xbig[:, 2:4, :], xsrc[:, 2:4, :])
            constb = cpool.tile([128, CB], BF16, name="constb", tag="constb")
            nc.sync.dma_start(constb[:], dcb[:])
            constf = cpool.tile([128, CF], F32, name="constf", tag="constf")
            nc.sync.dma_start(constf[:], dcf[:])
            Abig = wpool.tile([128, NT, N], F32R, name="Abig", tag="Abig")
            nc.sync.dma_start(Abig[:], dA.rearrange("(t p) c -> p t c", p=128))

            ident = constf[:, 0:128]
            ioffd = constf[:, 128:256]
            actb = constf[:, 256:259]
            eb2c = constf[:, 259:260]
            gbt = constf[0:F0, 260:263]
            fcwt = constf[0:F0, 263:265]
            fcbt = constf[0:OUT, 265:266]
            eb1c = constf[0:HID, 266:267]
            onesrow = constf[0:1, 269:397]  # [1,128] of ones
            identr = constr[:, 0:128]
            wab = [constr[:, 128 + 64 * c:128 + 64 * (c + 1)] for c in range(2)]
            gw0t = [constr[:, 256 + 64 * q:256 + 64 * (q + 1)] for q in range(4)]
            gw1t = constr[:, 512:576]
            gw2t = constr[:, 576:640]
            w2cv = [constb[:, 32 * u:32 * (u + 1)] for u in range(8)]
            identb = constb[:, 256:384]
            xt = [xbig[:, t, :] for t in range(NT)]
            At = [Abig[:, t, :] for t in range(NT)]

            # ---------------- xaT / xbT ----------------
            xT = [wpool.tile([128, N], F32R, name=f"xT{c2}", tag=f"xT{c2}")
                  for c2 in range(2)]
            for c2 in range(2):
                pb = ppool.tile([128, N], F32R, name="bank", tag="bank")
                for t in range(NT):
                    nc.tensor.transpose(
                        pb[:, 128 * t:128 * (t + 1)],
                        xt[t][:, 128 * c2:128 * (c2 + 1)],
                        identr[:],
                    )
                nc.scalar.activation(xT[c2][:], pb[:], AF.Copy)
            abT = ppool.tile([2 * HID, N], F32, name="bank", tag="bank")
            for c2 in range(2):
                nc.tensor.matmul(
                    abT[:], wab[c2], xT[c2][:],
                    start=(c2 == 0), stop=(c2 == 1),
                )
            xaT = wpool.tile([HID, N], F32, name="xaT", tag="xaT")
            nc.vector.tensor_scalar(xaT[:], abT[0:HID, :], eb1c, None, OP.add)
            B_all = wpool.tile([128, 128], F32, name="B_all", tag="B_all")
            for s in range(4):
                nc.vector.tensor_copy(B_all[32 * s:32 * (s + 1), :], xaT[:, s::4])
            xbT_rep = wpool.tile([128, N], BF16, name="xbT_rep", tag="xbT_rep")
            nc.scalar.activation(xbT_rep[0:HID, :], abT[HID:2 * HID, :], AF.Copy)
            for s in range(1, 4):
                nc.scalar.activation(
                    xbT_rep[32 * s:32 * (s + 1), :], xbT_rep[0:HID, :], AF.Copy
                )

            # ---------------- edge pass (+ interleaved A-side) ----------------
            # strip j=3 relus on ACT (issued first per i-tile, so the in-order
            # PE queue never waits on a late ACT tile); j=0..2 on DVE.
            d_all = wpool.tile([128, 8], F32, name="d_all", tag="d_all")
            AhT = [wpool.tile([128, N], F32R, name=f"AhT{jt}", tag=f"AhT{jt}")
                   for jt in range(NT)]

            def a_side(jt):
                ap_ = ppool.tile([128, N], F32R, name="bank", tag="bank")
                for it2 in range(NT):
                    nc.tensor.transpose(
                        ap_[:, 128 * it2:128 * (it2 + 1)],
                        At[it2][:, 128 * jt:128 * (jt + 1)],
                        identr[:],
                    )
                nc.vector.tensor_scalar(
                    AhT[jt][:], ap_[:], 0.0, 0.0, OP.add, OP.add,
                    accum_out=d_all[:, jt:jt + 1],
                )
                nc.gpsimd.tensor_tensor(
                    AhT[jt][:, 128 * jt:128 * (jt + 1)],
                    AhT[jt][:, 128 * jt:128 * (jt + 1)],
                    ident, OP.add,
                )

            # A^T for two j-tiles while DVE builds the edge-bias tables
            # (fills the PE bubble before the first edge matmul)
            a_side(0)
            a_side(1)

            lnd = wpool.tile([128, 8], F32, name="lnd", tag="lnd")
            D_all = wpool.tile([128, 8], F32, name="D_all", tag="D_all")
            Dt = wpool.tile([1, 8, 128], F32, name="Dt", tag="Dt")
            Dbc = [wpool.tile([128, N], F32, name=f"Dbc{r}", tag=f"Dbc{r}")
                   for r in range(2)]

            def d_path(r):
                lo = 4 * r
                nc.scalar.activation(lnd[:, lo:lo + 4], d_all[:, lo:lo + 4],
                                     AF.Ln, bias=actb[:, r:r + 1])
                nc.scalar.activation(D_all[:, lo:lo + 4], lnd[:, lo:lo + 4],
                                     AF.Exp, bias=actb[:, 2:3], scale=-0.5)
                dtp = ppool2.tile([1, 4, 128], F32, name="small", tag="small")
                for c4 in range(4):
                    nc.tensor.transpose(
                        dtp[0:1, c4, :], D_all[:, lo + c4:lo + c4 + 1], ident
                    )
                nc.vector.tensor_copy(Dt[0:1, lo:lo + 4, :], dtp[:])
                bp = ppool.tile([128, N], F32, name="bank", tag="bank")
                for jt in range(NT):
                    nc.tensor.matmul(
                        bp[:, 128 * jt:128 * (jt + 1)],
                        onesrow, Dt[0:1, lo + jt, :],
                        start=True, stop=True,
                    )
                nc.scalar.activation(Dbc[r][:], bp[:], AF.Copy)

            def edge_relus_j3(it):
                tiles = []
                for u in range(8):
                    g = 32 * it + 8 * 3 + u
                    t_a = tpool.tile([128, N], BF16, name="ta", tag="ta")
                    nc.scalar.activation(t_a[:], xbT_rep[:], AF.Relu,
                                         bias=B_all[:, g:g + 1])
                    tiles.append(t_a)
                return tiles

            L0T = [wpool.tile([128, N], F32R, name=f"L0T{jt}", tag=f"L0T{jt}")
                   for jt in range(NT)]
            L1T = [wpool.tile([128, N], F32R, name=f"L1T{jt}", tag=f"L1T{jt}")
                   for jt in range(NT)]
            e_sb = [wpool.tile([128, N], BF16, name=f"e{it}", tag=f"e{it}")
                    for it in range(NT)]

            eps = [None] * NT
            j3 = edge_relus_j3(0)
            for it in range(NT):
                ep = ppool.tile([128, N], F32, name="bank", tag="bank")
                eps[it] = ep
                for u in range(8):
                    for j in range(4):
                        g = 32 * it + 8 * j + u
                        if j == 3:
                            t_t = j3[u]
                        else:
                            t_t = tpool.tile([128, N], BF16, name="t", tag="t")
                            nc.vector.tensor_scalar(
                                t_t[:], xbT_rep[:], B_all[:, g:g + 1], 0.0,
                                OP.add, OP.max,
                            )
                        nc.tensor.matmul(
                            ep[32 * j:32 * (j + 1), :], w2cv[u], t_t[:],
                            start=(u == 0), stop=(u == 7),
                            tile_position=(0, 32 * j),
                        )
                # next i-tile's ACT relus BEFORE this tile's e_sb copy, so
                # the in-order ACT queue never stalls the next PE strip
                if it + 1 < NT:
                    j3 = edge_relus_j3(it + 1)
                nc.scalar.activation(e_sb[it][:], eps[it][:], AF.Copy)
                if it < 2:
                    a_side(2 + it)
                if it == 2:
                    # D0 chain mid-edge: its ACT ln/exp slots into the edge
                    # ACT stream here instead of queueing behind all of it;
                    # L0T built on the idle GPSIMD engine
                    d_path(0)
                    for jt in range(NT):
                        nc.vector.scalar_tensor_tensor(
                            L0T[jt][:], AhT[jt][:], D_all[:, jt:jt + 1],
                            Dbc[0][:], OP.mult, OP.mult,
                        )

            # ---------------- symmetrize, exp -> Y = A_pred, d1 -------------
            Yt = [wpool.tile([128, N], F32, name=f"Y{it}", tag=f"Y{it}")
                  for it in range(NT)]
            d1raw = wpool.tile([128, 4], F32, name="d1raw", tag="d1raw")
            diagv = wpool.tile([128, 4], F32, name="diagv", tag="diagv")
            for it in range(NT):
                etp = ppool.tile([128, N], BF16, name="etp", tag="bank")
                for jt in range(NT):
                    nc.tensor.transpose(
                        etp[:, 128 * jt:128 * (jt + 1)],
                        e_sb[jt][:, 128 * it:128 * (it + 1)],
                        identb,
                    )
                s_t = tpool.tile([128, N], BF16, name="t", tag="t")
                nc.vector.tensor_tensor(s_t[:], e_sb[it][:], etp[:], OP.add)
                nc.scalar.activation(Yt[it][:], s_t[:], AF.Exp, bias=eb2c,
                                     scale=0.5, accum_out=d1raw[:, it:it + 1])
                # extract the (wrong) diagonal, fix the diag block to
                # A_hat1 = Y_offdiag + I, and correct d1 = d1raw - diag
                # (the +1 from I is folded into the Ln bias)
                dblk = Yt[it][:, 128 * it:128 * (it + 1)]
                scr = tpool.tile([128, 128], F32, name="scr", tag="scr", bufs=2)
                nc.vector.scalar_tensor_tensor(
                    scr[:], dblk, 1.0, ident, OP.mult, OP.mult,
                    accum_out=diagv[:, it:it + 1],
                )
                nc.vector.tensor_tensor(dblk, dblk, scr[:], OP.subtract)
                nc.vector.tensor_tensor(dblk, dblk, ident, OP.add)
                nc.vector.tensor_tensor(
                    d_all[:, 4 + it:5 + it], d1raw[:, it:it + 1],
                    diagv[:, it:it + 1], OP.subtract,
                )

            # ---------------- GCN layer 0, relation 0 (overlaps L1T build) --
            xh0 = [wpool.tile([128, N], F32R, name=f"xh0_{q}", tag=f"xh0_{q}")
                   for q in range(4)]
            up0 = []
            for c2 in range(2):
                up = ppool.tile([128, N], F32, name="bank", tag="bank")
                for jt in range(NT):
                    nc.tensor.matmul(
                        up[:], xt[jt][:, 128 * c2:128 * (c2 + 1)], L0T[jt][:],
                        start=(jt == 0), stop=(jt == NT - 1),
                    )
                nc.scalar.activation(xh0[c2][:], up[:], AF.Copy)

            d_path(1)
            for jt in range(NT):
                nc.vector.scalar_tensor_tensor(
                    L1T[jt][:], Yt[jt][:], D_all[:, 4 + jt:5 + jt], Dbc[1][:],
                    OP.mult, OP.mult,
                )

            # ---------------- GCN layer 0, relation 1 + dense ----------------
            for c2 in range(2):
                up = ppool.tile([128, N], F32, name="bank", tag="bank")
                for jt in range(NT):
                    nc.tensor.matmul(
                        up[:], xt[jt][:, 128 * c2:128 * (c2 + 1)], L1T[jt][:],
                        start=(jt == 0), stop=(jt == NT - 1),
                    )
                nc.scalar.activation(xh0[2 + c2][:], up[:], AF.Copy)
            zp = ppool.tile([F0, N], F32, name="bank", tag="bank")
            for q in range(4):
                nc.tensor.matmul(
                    zp[:], gw0t[q], xh0[q][:],
                    start=(q == 0), stop=(q == 3),
                )
            h1T = wpool.tile([F0, N], F32R, name="h1T", tag="h1T")
            nc.scalar.activation(h1T[:], zp[:], AF.Relu, bias=gbt[:, 0:1])

            # ---------------- GCN layers 1, 2 ----------------
            hT = h1T
            for li in (1, 2):
                h_sb = [wpool.tile([128, F0], F32R, name=f"h{li}_{jt}",
                                   tag=f"h{li}_{jt}") for jt in range(NT)]
                for jt in range(NT):
                    hp = ppool2.tile([128, F0], F32R, name="small", tag="small")
                    nc.tensor.transpose(
                        hp[:], hT[:, 128 * jt:128 * (jt + 1)],
                        identr[0:F0, 0:F0],
                    )
                    nc.vector.tensor_copy(h_sb[jt][:], hp[:])
                xh1 = wpool.tile([128, N], F32R, name=f"xh{li}", tag=f"xh{li}")
                for r in range(2):
                    up = ppool.tile([F0, N], F32, name="bank", tag="bank")
                    LrT = L0T if r == 0 else L1T
                    for jt in range(NT):
                        nc.tensor.matmul(
                            up[:], h_sb[jt][:], LrT[jt][:],
                            start=(jt == 0), stop=(jt == NT - 1),
                        )
                    nc.scalar.activation(xh1[F0 * r:F0 * (r + 1), :], up[:],
                                         AF.Copy)
                zp2 = ppool.tile([F0, N], F32, name="bank", tag="bank")
                gwt = gw1t if li == 1 else gw2t
                nc.tensor.matmul(zp2[:], gwt, xh1[:], start=True, stop=True)
                houtT = wpool.tile([F0, N], F32R, name=f"h{li}T", tag=f"h{li}T")
                nc.scalar.activation(houtT[:], zp2[:], AF.Relu,
                                     bias=gbt[:, li:li + 1])
                hT = houtT

            # ---------------- pool + classifier ----------------
            gmax = wpool.tile([F0, 1], F32, name="gmax", tag="gmax")
            nc.vector.tensor_reduce(gmax[:], hT[:], axis=AX.X, op=OP.max)
            op_ = ppool2.tile([OUT, 1], F32, name="small", tag="small")
            nc.tensor.matmul(op_[:], fcwt, gmax[:], start=True, stop=True)
            obuf = wpool.tile([OUT, 1], F32, name="obuf", tag="obuf")
            nc.vector.tensor_scalar(obuf[:], op_[:], fcbt, None, OP.add)
            nc.sync.dma_start(dout[:], obuf[:])

    _legalize_waits(nc)
    nc.finalize()
    return nc


_CACHE: dict = {}


def _get_compiled():
    if "nc" not in _CACHE:
        _CACHE["nc"] = build_kernel()
    return _CACHE["nc"]


def _pack_consts(ew1, eb1, ew2, eb2, gw0, gb0, gw1, gb1, gw2, gb2, fcw, fcb):
    bf = ml_dtypes.bfloat16
    f32 = np.float32
    ident = np.eye(128, dtype=f32)

    constf = np.zeros((128, CF), f32)
    constf[:, 0:128] = ident
    constf[:, 128:256] = 1.0 - ident
    constf[:, 256] = 1.0 + EPS
    constf[:, 257] = 1.0 + EPS
    constf[:, 258] = 0.0
    constf[:, 259] = float(np.asarray(eb2).reshape(-1)[0])
    constf[0:F0, 260] = np.asarray(gb0, f32)
    constf[0:F0, 261] = np.asarray(gb1, f32)
    constf[0:F0, 262] = np.asarray(gb2, f32)
    constf[0:F0, 263:265] = np.asarray(fcw, f32)
    constf[0:OUT, 265] = np.asarray(fcb, f32).reshape(-1)
    constf[0:HID, 266] = np.asarray(eb1, f32)
    constf[:, 267] = 1.0
    constf[:, 268] = 1.0
    constf[0, 269:397] = 1.0

    constr = np.zeros((128, CR), f32)
    constr[:, 0:128] = ident
    wabf = np.concatenate([ew1[:C], ew1[C:]], axis=1).astype(f32)  # [256, 64]
    constr[:, 128:192] = wabf[0:128]
    constr[:, 192:256] = wabf[128:256]
    g0 = np.asarray(gw0, f32)
    for q in range(4):
        constr[:, 256 + 64 * q:256 + 64 * (q + 1)] = g0[128 * q:128 * (q + 1)]
    constr[:, 512:576] = np.asarray(gw1, f32)
    constr[:, 576:640] = np.asarray(gw2, f32)

    constb = np.zeros((128, CB), f32)
    w2 = np.asarray(ew2[:, 0], f32)
    for s in range(4):
        for u in range(8):
            constb[32 * s:32 * (s + 1), 32 * u + 4 * u + s] = w2
    constb[:, 256:384] = ident
    return dict(
        constf=constf,
        constr=constr,
        constb=constb.astype(bf),
    )


def kernel(x, A, mask, ew1, eb1, ew2, eb2, gw0, gb0, gw1, gb1, gw2, gb2,
           fcw, fcb, _trace=False, **_ignored):
    nc = _get_compiled()
    consts = _pack_consts(ew1, eb1, ew2, eb2, gw0, gb0, gw1, gb1, gw2, gb2,
                          fcw, fcb)
    in_maps = [
        dict(
            consts,
            x=np.ascontiguousarray(np.asarray(x)[b], np.float32),
            A=np.ascontiguousarray(np.asarray(A)[b], np.float32),
        )
        for b in range(B)
    ]
    res = run_bass_kernel_spmd(
        nc, in_maps, core_ids=list(range(B)), trace=_trace
    )
    out = np.stack(
        [res.results[b]["out"].reshape(OUT) for b in range(B)]
    ).astype(np.float32)
    if _trace:
        kernel.last_results = res
    return out
